# revision 1
# baseline (speedup 1.0000x reference)
"""Trainium2 Bass kernel for the ANFIS forward pass (8-core data-parallel).

Math: with L[b,f,m] = -0.5*((X[b,f]-mu[f,m])/sigma[f,m])^2,
  miAlloc[b,r] = prod_f exp(L[b,f,rules[r,f]])
  out[b] = (miAlloc @ c) / (sum_r miAlloc + 1e-10),  c = consequents.sum(1)

Factor the 8 features into two halves of 4. Each half has 81 possible
membership tuples, so miAlloc[b,r] = W1[b,rho1(r)] * W2[b,rho2(r)] where
  W1[b,t] = exp(sum_{f<4} a[f,tf]*(X[b,f]-mu[f,tf])^2),  a = -0.5/sigma^2
and rho1/rho2 map each rule to its half-tuple index. With
  C2[t1,t2] = sum_{r: rho(r)=(t1,t2)} c[r],   D2[t1,t2] = #{r: rho(r)=(t1,t2)}
(exact for arbitrary `rules`, duplicates included):
  num[b] = sum_{t2} (C2^T W1)[t2,b] * W2[t2,b]
  den[b] = sum_{t2} (D2^T W1)[t2,b] * W2[t2,b]
  out[b] = num[b] / (den[b] + 1e-10)     <- divide happens on HOST

Device-side design notes (all fp16 data path, fp32 PSUM accumulation):
 * logW is computed as a single K=18 matmul over z = [x(8) | x^2(8) | 1 | 1]:
   a*(x-mu)^2 = a*x^2 - 2*a*mu*x + a*mu^2. The quadratic x^2 rows are squared
   in place on VectorE (2-byte 2x mode); the two ones-rows carry the constant
   term split hi/lo across two fp16 rows so it lands with ~fp32 precision.
 * exp() values are scaled by e^SHIFT per half to stay out of fp16 subnormals;
   the scale cancels in num/den (host divides with a rescaled epsilon).
 * The PE p-state ramps 0.65->1.2->2.4 GHz with ~3us of continuous work, so
   a run of warm-up matmuls on garbage SBUF keeps the array busy through the
   framework preamble + input DMA flight; real matmuls then run at full rate.
 * num/den are reduced by ones-matmuls into one [64,512] PSUM tile per half,
   copied to SBUF on ScalarE (DMA cannot read PSUM), and DMA'd out as two
   rows; the final divide is elementwise host post-processing of the gather.
"""

import numpy as np

import concourse.bass as bass
import concourse.tile as tile
from concourse import bacc, mybir
from concourse.bass_utils import run_bass_kernel_spmd

B, F, M = 8192, 8, 3
NC = 8
BC = B // NC  # 1024 batch rows per core
HB = BC // 2  # 512-column half
T = M**4  # 81 tuples per feature-half
K = 18  # x(8) | x^2(8) | ones(2)
FP32 = mybir.dt.float32
FP16 = mybir.dt.float16
AF = mybir.ActivationFunctionType
SHIFT = 2.0  # per-half exp scale; cancels in num/den
N_WARM = 6  # PE p-state warm-up matmuls (cold->mid ramp; HW won't boost past 1.2GHz)

_CACHE = {}


def _build_graph(sep):
    """sep=True: D2 is rank-1 (u v^T / s), den computed as (u^T W1)(v^T W2)/s
    on the host from shipped s1/s2 rows. sep=False: general D2 path with the
    hd matmuls + pd muls + den reduce on device."""
    nc = bacc.Bacc("TRN2", target_bir_lowering=False, debug=False, num_devices=NC)

    # xqw: batch-half h0 | stage-1 weights A1,A2 | batch-half h1. Embedding
    # the weights in the h0 transfer delivers them with the first DMA instead
    # of gating the first matmul on a separate (late) weights DMA.
    XW = BC + 2 * T  # 1186 columns
    xqw_ext = nc.dram_tensor("xqw", [K, XW], FP16, kind="ExternalInput").ap()
    # bigc: sep: C2 | ones,u,v [81, 84]; general: C2 | D2 | ones [81, 163]
    BW = 2 * T + 1 if not sep else T + 3
    bigc_ext = nc.dram_tensor("bigc", [T, BW], FP16, kind="ExternalInput").ap()
    # o rows (cols = h0|h1): sep: num, s1, s2; general: num, den
    out_ext = nc.dram_tensor("o", [3 if sep else 2, BC], FP32, kind="ExternalOutput").ap()

    with tile.TileContext(nc) as tc:
        with (
            tc.tile_pool(name="const", bufs=1) as const,
            tc.tile_pool(name="work", bufs=1) as work,
            tc.tile_pool(name="psum", bufs=1, space=bass.MemorySpace.PSUM) as psum,
        ):
            xqw = const.tile([K, XW], FP16)
            xh = [xqw[:, 0:HB], xqw[:, HB + 2 * T : XW]]
            wb = xqw[:, HB : HB + 2 * T]
            bigc = const.tile([T, BW], FP16)
            c2 = bigc[:, 0:T]
            if sep:
                ones1 = bigc[:, T : T + 1]
                ucol = bigc[:, T + 1 : T + 2]
                vcol = bigc[:, T + 2 : T + 3]
            else:
                d2 = bigc[:, T : 2 * T]
                ones1 = bigc[:, 2 * T : 2 * T + 1]

            # input DMAs on the sync HWDGE queue: h0+weights first (gates
            # everything), then h1; C2 etc. on the gpsimd SWDGE queue
            # (needed ~2.5us after trigger, latency hidden)
            nc.sync.dma_start(
                out=xqw[:, 0 : HB + 2 * T],
                in_=xqw_ext[:, 0 : HB + 2 * T],
                single_packet=True,
            )
            nc.sync.dma_start(
                out=xqw[:, HB + 2 * T : XW],
                in_=xqw_ext[:, HB + 2 * T : XW],
                single_packet=True,
            )
            nc.gpsimd.dma_start(out=bigc[:, :], in_=bigc_ext[:, :])

            # PSUM: 8 banks, tags reused once the lw tiles are consumed
            warm = psum.tile([T, HB], FP32, tag="pc", name="warm")
            lw = [
                psum.tile([T, HB], FP32, tag=t, name=f"lw{t}")
                for t in ("pa", "pb", "pc", "pd")
            ]  # w1h0, w2h0, w1h1, w2h1
            ht = [psum.tile([T, HB], FP32, tag=t, name=f"ht{t}") for t in ("pe", "pf")]
            if not sep:
                hd = [
                    psum.tile([T, HB], FP32, tag=t, name=f"hd{t}")
                    for t in ("pg", "ph")
                ]
            # nd rows used: 0 = s1/den, 32 = s2 (sep only), NUMR = num.
            # Dedicated banks (pg/ph free in sep mode) -- aliasing these onto
            # the lw banks made the tile scheduler emit conservative waits.
            ndrows = 96 if sep else 64
            NUMR = 64 if sep else 32
            ndtags = ("pg", "ph") if sep else ("pa", "pb")
            nd = [
                psum.tile([ndrows, HB], FP32, tag=t, name=f"nd{t}") for t in ndtags
            ]

            w = work.tile([T, 2 * BC], FP16)  # w1 cols 0:BC, w2 cols BC:2BC
            p = work.tile([T, 2 * BC], FP16)  # p1h0 | pdh0 | p1h1 | pdh1
            cprows = ndrows - 31
            # outt rows 0=s1/den, 32=s2 (sep), NUMR=num; cols h0|h1
            outt = work.tile([cprows, BC], FP32)
            warm_l = work.tile([K, T], FP16)

            # PE warm-up: gated only on a tiny vector memset, so it runs from
            # the branch into the kernel body, covering the cold->mid ramp
            nc.vector.memset(warm_l[:, :], 0.0)
            for _ in range(N_WARM):
                nc.tensor.matmul(warm[:, 0:T], lhsT=warm_l[:, :], rhs=warm_l[:, :])

            w1 = [w[:, bass.ts(h, HB)] for h in range(2)]
            w2 = [w[:, bass.ds(BC + h * HB, HB)] for h in range(2)]
            for h in range(2):
                nc.tensor.matmul(lw[2 * h][:, :], lhsT=wb[:, 0:T], rhs=xh[h])
                nc.tensor.matmul(lw[2 * h + 1][:, :], lhsT=wb[:, T : 2 * T], rhs=xh[h])
                nc.scalar.activation(w1[h], lw[2 * h][:, :], AF.Exp)
                nc.scalar.activation(w2[h], lw[2 * h + 1][:, :], AF.Exp)
            # PE order: ht0 then ht1 ahead of the reduce matmuls -- ht1 is on
            # the critical tail, while the s/rn single-row matmuls overlap it
            # (and each other) in distinct PE column groups
            nc.tensor.matmul(ht[0][:, :], lhsT=c2, rhs=w1[0])
            nc.tensor.matmul(ht[1][:, :], lhsT=c2, rhs=w1[1])
            for h in range(2):
                nc.vector.tensor_mul(p[:, bass.ts(2 * h, HB)], ht[h][:, :], w2[h])
            # nd rows: 0 = s1/den, 32 = s2 (sep), 64 = num -- the early rows
            # sit in the base-0 window; engine access patterns with a
            # non-zero partition base may span at most 32 partitions.
            for h in range(2):
                if sep:
                    nc.tensor.matmul(nd[h][0:1, :], lhsT=ucol, rhs=w1[h])
                    nc.tensor.matmul(nd[h][32:33, :], lhsT=vcol, rhs=w2[h])
                else:
                    nc.tensor.matmul(hd[h][:, :], lhsT=d2, rhs=w1[h])
                    nc.vector.tensor_mul(
                        p[:, bass.ts(2 * h + 1, HB)], hd[h][:, :], w2[h]
                    )
                    nc.tensor.matmul(
                        nd[h][0:1, :], lhsT=ones1, rhs=p[:, bass.ts(2 * h + 1, HB)]
                    )
                nc.tensor.matmul(
                    nd[h][NUMR : NUMR + 1, :], lhsT=ones1, rhs=p[:, bass.ts(2 * h, HB)]
                )
            # copies emitted after the whole compute loop so the scalar queue
            # runs all four exps before any copy; one [cprows,512] copy per
            # half (h0 on Scalar, h1 on Vector) so the two overlap
            nc.scalar.copy(outt[:, 0:HB], nd[0][0:cprows, :])
            nc.vector.tensor_copy(outt[:, HB:BC], nd[1][0:cprows, :])

            nc.sync.dma_start(out=out_ext[:, :], in_=outt[0:cprows:32, :])

    nc.compile()
    return nc


def _get_graph(sep):
    key = f"nc{int(sep)}"
    if key not in _CACHE:
        _CACHE[key] = _build_graph(sep)
    return _CACHE[key]


def _prep_inputs(X, mu, sigma, consequents, rules):
    X = np.ascontiguousarray(np.asarray(X, dtype=np.float32))
    mu64 = np.asarray(mu, dtype=np.float64)
    c = np.asarray(consequents, dtype=np.float64).sum(axis=1)
    r = np.asarray(rules).astype(np.int64)

    a = -0.5 / (np.asarray(sigma, np.float64) ** 2)  # [F, M]

    # tuple digit j of t (digit 0 most significant), t in [0, 81)
    digits = (np.arange(T)[:, None] // np.array([27, 9, 3, 1])[None, :]) % 3  # [81, 4]

    # A[half]: rows 0:8 coeff for x^2 rows, 8:16 for x rows, 16:18 the
    # constant term split hi/lo (the matching xq rows are 1.0)
    wb = np.zeros((K, 2 * T), np.float16)
    for half in range(2):
        A = np.zeros((16, T), np.float64)
        b = np.full(T, SHIFT, np.float64)
        for j in range(4):
            f = 4 * half + j
            d = digits[:, j]
            A[f, :] = a[f, d]
            A[8 + f, :] = -2.0 * a[f, d] * mu64[f, d]
            b += a[f, d] * mu64[f, d] ** 2
        wb[0:16, half * T : (half + 1) * T] = A.astype(np.float16)
        b_hi = b.astype(np.float16)
        b_lo = (b - b_hi.astype(np.float64)).astype(np.float16)
        wb[16, half * T : (half + 1) * T] = b_hi
        wb[17, half * T : (half + 1) * T] = b_lo

    rho1 = ((r[:, 0] * 3 + r[:, 1]) * 3 + r[:, 2]) * 3 + r[:, 3]
    rho2 = ((r[:, 4] * 3 + r[:, 5]) * 3 + r[:, 6]) * 3 + r[:, 7]
    C2 = np.zeros((T, T), np.float64)
    np.add.at(C2, (rho1, rho2), c)
    D2 = np.zeros((T, T), np.float64)
    np.add.at(D2, (rho1, rho2), 1.0)

    # Separable den path when D2 is rank-1 with fp16-exact factors (true for
    # the reference's full cartesian-product rules: D2 is all-ones).
    u = D2.sum(axis=1)
    v = D2.sum(axis=0)
    s = D2.sum()
    sep = (
        s > 0
        and np.array_equal(np.outer(u, v) / s, D2 * 1.0)
        and np.array_equal(u.astype(np.float16).astype(np.float64), u)
        and np.array_equal(v.astype(np.float16).astype(np.float64), v)
    )
    _CACHE["sep"] = sep
    _CACHE["dscale"] = s

    if sep:
        bigc = np.zeros((T, T + 3), np.float16)
        bigc[:, 0:T] = C2.astype(np.float16)
        bigc[:, T] = 1.0
        bigc[:, T + 1] = u.astype(np.float16)
        bigc[:, T + 2] = v.astype(np.float16)
    else:
        bigc = np.zeros((T, 2 * T + 1), np.float16)
        bigc[:, 0:T] = C2.astype(np.float16)
        bigc[:, T : 2 * T] = D2.astype(np.float16)
        bigc[:, 2 * T] = 1.0
    bigc = np.ascontiguousarray(bigc)

    Xsh = X.reshape(NC, BC, F)
    xqw = np.empty((NC, K, BC + 2 * T), np.float16)  # xh0 | A1,A2 | xh1
    xt = np.swapaxes(Xsh, 1, 2)  # [NC, F, BC] fp32
    for blk, s in ((slice(0, HB), slice(0, HB)), (slice(HB + 2 * T, None), slice(HB, BC))):
        xqw[:, 0:8, blk] = (xt[:, :, s] ** 2).astype(np.float16)
        xqw[:, 8:16, blk] = xt[:, :, s].astype(np.float16)
        xqw[:, 16:18, blk] = 1.0
    xqw[:, :, HB : HB + 2 * T] = wb[None, :, :]

    in_maps = [{"xqw": np.ascontiguousarray(xqw[i]), "bigc": bigc} for i in range(NC)]
    return in_maps


def _run(in_maps, trace=False, **kwargs):
    nc = _get_graph(_CACHE.get("sep", True))
    return run_bass_kernel_spmd(
        nc, in_maps, core_ids=list(range(NC)), trace=trace, **kwargs
    )


def kernel(X, mu, sigma, consequents, rules):
    in_maps = _prep_inputs(X, mu, sigma, consequents, rules)
    res = _run(in_maps)
    eps = np.float32(1e-10 * np.exp(2.0 * SHIFT))
    outs = []
    if _CACHE["sep"]:
        ds = np.float32(1.0 / _CACHE["dscale"])
        for i in range(NC):
            # rows: s1, s2, num
            o = np.asarray(res.results[i]["o"], dtype=np.float32)  # [3, BC]
            outs.append(o[2] / (o[0] * o[1] * ds + eps))
    else:
        for i in range(NC):
            # rows: den, num
            o = np.asarray(res.results[i]["o"], dtype=np.float32)  # [2, BC]
            outs.append(o[1] / (o[0] + eps))
    return np.concatenate(outs).astype(np.float32)

